# revision 1
# baseline (speedup 1.0000x reference)
"""nn_Attention multi-head attention on 8 TRN2 NeuronCores.

Sharding (no collectives): core c handles batch b=c//2 and query-half
qh=c%2 (1024 query tokens). Each core computes the QKV projection for its
batch (K,V for all 2048 tokens, duplicated across the pair of cores that
share a batch), attention for all 16 heads over its 1024 queries, and the
output projection for its tokens. The host concatenates the 8 disjoint
[1024, 1024] output slices.

Device-side structure (per core):
  - inputs fed as bf16: x[b].T (this core's query half first; key order is a
    consistent permutation of K and V so attention is unchanged), w_qkv.T,
    w_proj.T, b_proj
  - attention computed in transposed layout S^T = K_h Q_h^T per 128-key
    tile; softmax denominators via an all-ones column appended to V; exp on
    the Scalar engine straight out of PSUM; all matmuls bf16 with fp32 PSUM
  - K^T/Q^T are duplicated into both partition halves so consecutive
    score matmuls alternate PE row groups (weight-load pull-ahead)
  - QKV projection chunks of the NEXT head-pair and output-projection
    partials of the PREVIOUS pair are emission-interleaved into the
    attention key-tile loop so the TensorEngine stays dense; the output
    projection accumulates per-pair partials in an SBUF f32 buffer
"""

import contextlib

import numpy as np
import orjson

import concourse.bass as bass
import concourse.mybir as mybir
import concourse.tile as tile
from concourse.vector_clock import ScopedClock

# ---------------------------------------------------------------------------
# Workarounds for the walrus build in this container, which accepts at most
# one sync wait per engine instruction (two for EventSemaphore):
#  1. Tile's end-of-kernel drain carries one wait per outstanding semaphore --
#     redistribute over a chain of sync-engine NOPs.
#  2. Tile's scheduler also emits multi-wait body instructions -- split them
#     in the serialized BIR by inserting same-engine NOPs ahead of the
#     offender (engine program order makes the chain equivalent).
# ---------------------------------------------------------------------------


def _patched_drain_and_barrier(self, tick_clock, wait_clock):
    nc = self.nc
    collector = nc.sync.nop()
    wait_clock.add_sem_waits(
        collector.ins, ScopedClock({None: tick_clock.global_clock})
    )
    si = collector.ins.sync_info
    waits = list(si.on_wait or []) if si is not None else []
    if si is not None:
        si.on_wait = waits[:1]
    import bass_rust as _br

    for w in waits[1:]:
        n = nc.sync.nop()
        n.ins.sync_info = _br.SyncInfo(on_wait=[w], on_update=[])

    nc.sync.drain()
    nc.all_engine_barrier()
    assert self.sems is not None
    popped = nc._tile_sem_poison_stack.pop()
    assert popped is self._sem_poison
    nc.clear_and_free_semaphores(list(self.sems.allocated().values()))
    nc.all_engine_barrier()


_WCAPS = {"EventSemaphore": 2}
_wcounter = [0]


def _split_waits_json(bir_bytes: bytes) -> bytes:
    j = orjson.loads(bir_bytes)
    changed_any = False
    for f in j.get("functions", []):
        for b in f.get("blocks", []):
            outl = []
            changed = False
            for ins in b["instructions"]:
                si = ins.get("sync_info")
                waits = (si or {}).get("on_wait") or []
                cap = _WCAPS.get(ins.get("opcode"), 1)
                engine = ins.get("engine")
                if len(waits) > cap and engine and engine != "Unassigned":
                    changed = True
                    extra, keep = waits[:-cap], waits[-cap:]
                    for w in extra:
                        _wcounter[0] += 1
                        outl.append({
                            "name": f"I-wsplit-{_wcounter[0]}",
                            "opcode": "NoOp",
                            "engine": engine,
                            "ins": [],
                            "outs": [],
                            "sync_info": {"on_update": [], "on_wait": [w]},
                        })
                    si["on_wait"] = keep
                outl.append(ins)
            if changed:
                b["instructions"] = outl
                changed_any = True
    return orjson.dumps(j) if changed_any else bir_bytes


def _apply_patches():
    if not getattr(tile.TileContext, "_attn_drain_patched", False):
        tile.TileContext._drain_and_barrier = _patched_drain_and_barrier
        tile.TileContext._attn_drain_patched = True
    if not getattr(bass.Bass, "_attn_wait_split_patched", False):
        orig = bass.Bass.to_json_bytes

        def to_json_bytes(self, *a, **kw):
            return _split_waits_json(orig(self, *a, **kw))

        bass.Bass.to_json_bytes = to_json_bytes
        bass.Bass._attn_wait_split_patched = True


F32 = mybir.dt.float32
BF16 = mybir.dt.bfloat16

C = 1024
H = 16
HD = 64
NK = 2048
NQ = 1024
SCALE = HD ** -0.5
KT_TILES = NK // 128
CT_TILES = C // 128
VSLOT = HD + 1


def build_nc():
    _apply_patches()
    nc = bass.Bass("TRN2", num_devices=8)
    xt = nc.declare_dram_parameter("xt", [C, NK], BF16, isOutput=False)
    wqkvt = nc.declare_dram_parameter("wqkvt", [C, 3 * C], BF16, isOutput=False)
    wpt = nc.declare_dram_parameter("wpt", [C, C], BF16, isOutput=False)
    bias = nc.declare_dram_parameter("bias", [1, C], BF16, isOutput=False)
    out = nc.declare_dram_parameter("out", [NQ, C], F32, isOutput=True)

    with tile.TileContext(nc) as tc:
        with contextlib.ExitStack() as es:
            persist = es.enter_context(tc.tile_pool(name="persist", bufs=1))
            ones = persist.tile([1, 128], BF16, tag="ones")
            nc.vector.memset(ones[:], 1.0)
            ident = persist.tile([128, 64], BF16, tag="ident")
            nc.gpsimd.memset(ident[0:64, :], 0.0)
            nc.gpsimd.affine_select(
                out=ident[0:64, :], in_=ident[0:64, :],
                compare_op=mybir.AluOpType.not_equal, fill=1.0,
                base=0, pattern=[[-1, 64]], channel_multiplier=1,
            )
            nc.sync.dma_start(out=ident[64:128, :], in_=ident[0:64, :])
            bias_sb = persist.tile([1, C], BF16, tag="bias")
            nc.sync.dma_start(out=bias_sb[:], in_=bias[:])

            nts = [persist.tile([128, NQ], BF16, tag=f"nt{ct}", name=f"nt{ct}")
                   for ct in range(CT_TILES)]
            vprimes = [persist.tile([128, 2 * KT_TILES * VSLOT], BF16,
                                    tag=f"vp{i}", name=f"vp{i}") for i in range(2)]
            for v in vprimes:
                nc.vector.memset(v[:], 1.0)
            oaccs = [persist.tile([128, C], F32, tag=f"oa{tt}", name=f"oa{tt}")
                     for tt in range(NQ // 128)]
            wp_pool = es.enter_context(tc.tile_pool(name="wp", bufs=CT_TILES))
            wpts = [wp_pool.tile([128, C], BF16, tag="wpt", name=f"wpts{ct}")
                    for ct in range(CT_TILES)]

            psum_mm = es.enter_context(tc.tile_pool(name="psum_mm", bufs=2, space="PSUM"))
            psum_s = es.enter_context(tc.tile_pool(name="psum_s", bufs=2, space="PSUM"))
            psum_u = es.enter_context(tc.tile_pool(name="psum_u", bufs=1, space="PSUM"))

            with contextlib.ExitStack() as es_attn:
                xt_pool = es_attn.enter_context(tc.tile_pool(name="xtp", bufs=CT_TILES))
                wq_pool = es_attn.enter_context(tc.tile_pool(name="wq", bufs=2))
                kt_pool = es_attn.enter_context(tc.tile_pool(name="kt", bufs=2))
                qt_pool = es_attn.enter_context(tc.tile_pool(name="qt", bufs=2))
                vstage_pool = es_attn.enter_context(tc.tile_pool(name="vstage", bufs=2))
                exp_pool = es_attn.enter_context(tc.tile_pool(name="exp", bufs=3))
                rsb_pool = es_attn.enter_context(tc.tile_pool(name="rsb", bufs=2))
                ktd_pool = es_attn.enter_context(tc.tile_pool(name="ktd", bufs=4))
                qtd_pool = es_attn.enter_context(tc.tile_pool(name="qtd", bufs=4))

                xts = []
                for ct in range(CT_TILES):
                    t = xt_pool.tile([128, NK], BF16, tag="xt", name=f"xts{ct}")
                    nc.sync.dma_start(out=t[:], in_=xt[ct * 128:(ct + 1) * 128, :])
                    xts.append(t)

                def prepare_qkv(p):
                    """Allocate tiles + DMA weights for pair p; return
                    (qt_sb, kt_sb, [chunk thunks])."""
                    w_sb = wq_pool.tile([128, 3 * CT_TILES * 128], BF16,
                                        tag="w", name=f"w{p}")
                    for m in range(3):
                        base = m * C + p * 128
                        for ct in range(CT_TILES):
                            o = (m * CT_TILES + ct) * 128
                            nc.sync.dma_start(
                                out=w_sb[:, o:o + 128],
                                in_=wqkvt[ct * 128:(ct + 1) * 128, base:base + 128],
                            )

                    def w_slice(m, ct):
                        o = (m * CT_TILES + ct) * 128
                        return w_sb[:, o:o + 128]

                    qt_sb = qt_pool.tile([128, NQ], BF16, tag="qt", name=f"qt{p}")
                    kt_sb = kt_pool.tile([128, NK], BF16, tag="kt", name=f"kt{p}")
                    ktd = [ktd_pool.tile([128, NK], BF16, tag="ktd", name=f"ktd{p}_{h}")
                           for h in range(2)]
                    qtd = [qtd_pool.tile([128, NQ], BF16, tag="qtd", name=f"qtd{p}_{h}")
                           for h in range(2)]
                    vp = vprimes[p % 2]
                    thunks = []

                    def q_chunk(tch):
                        def f():
                            ps = psum_mm.tile([128, 512], F32, tag="mm", name="psq")
                            for ct in range(CT_TILES):
                                nc.tensor.matmul(
                                    ps[:], w_slice(0, ct),
                                    xts[ct][:, tch * 512:(tch + 1) * 512],
                                    start=(ct == 0), stop=(ct == CT_TILES - 1),
                                )
                            csl = slice(tch * 512, (tch + 1) * 512)
                            nc.vector.tensor_copy(qt_sb[:, csl], ps[:])
                            for h2 in range(2):
                                hs = slice(h2 * 64, (h2 + 1) * 64)
                                nc.sync.dma_start(out=qtd[h2][0:64, csl],
                                                  in_=qt_sb[hs, csl])
                                nc.sync.dma_start(out=qtd[h2][64:128, csl],
                                                  in_=qt_sb[hs, csl])
                        return f

                    def k_chunk(tch):
                        def f():
                            ps = psum_mm.tile([128, 512], F32, tag="mm", name="psk")
                            for ct in range(CT_TILES):
                                nc.tensor.matmul(
                                    ps[:], w_slice(1, ct),
                                    xts[ct][:, tch * 512:(tch + 1) * 512],
                                    start=(ct == 0), stop=(ct == CT_TILES - 1),
                                )
                            csl = slice(tch * 512, (tch + 1) * 512)
                            nc.vector.tensor_copy(kt_sb[:, csl], ps[:])
                            for h2 in range(2):
                                hs = slice(h2 * 64, (h2 + 1) * 64)
                                nc.sync.dma_start(out=ktd[h2][0:64, csl],
                                                  in_=kt_sb[hs, csl])
                                nc.sync.dma_start(out=ktd[h2][64:128, csl],
                                                  in_=kt_sb[hs, csl])
                        return f

                    def v_chunk(tch):
                        def f():
                            ps = psum_mm.tile([128, 512], F32, tag="mm", name="psv")
                            for ct in range(CT_TILES):
                                nc.tensor.matmul(
                                    ps[:], w_slice(2, ct),
                                    xts[ct][:, tch * 512:(tch + 1) * 512],
                                    start=(ct == 0), stop=(ct == CT_TILES - 1),
                                )
                            vs = vstage_pool.tile([128, 512], BF16, tag="vs")
                            nc.vector.tensor_copy(vs[:], ps[:])
                            for h2 in range(2):
                                for sub in range(4):
                                    kt_idx = tch * 4 + sub
                                    pt = psum_mm.tile([128, 512], BF16,
                                                      tag="mm", name="pt")
                                    nc.tensor.matmul(
                                        pt[:, 0:64],
                                        vs[h2 * 64:(h2 + 1) * 64,
                                           sub * 128:(sub + 1) * 128],
                                        ident[h2 * 64:(h2 + 1) * 64, :],
                                        is_transpose=True,
                                    )
                                    slot = (h2 * KT_TILES + kt_idx) * VSLOT
                                    nc.vector.tensor_copy(
                                        vp[:, slot:slot + HD], pt[:, 0:64])
                        return f

                    # K and V first (attention needs full K/V; Q cheap last)
                    for tch in range(NK // 512):
                        thunks.append(k_chunk(tch))
                    for tch in range(NK // 512):
                        thunks.append(v_chunk(tch))
                    for tch in range(NQ // 512):
                        thunks.append(q_chunk(tch))
                    return qt_sb, kt_sb, ktd, qtd, thunks

                # prologue: pair 0 QKV fully
                qt_sb, kt_sb, ktd, qtd, thunks = prepare_qkv(0)
                for t in thunks:
                    t()

                def proj_partial_thunks(p):
                    thunks = []
                    for tt in range(NQ // 128):
                        for oc in range(C // 512):
                            def f(tt=tt, oc=oc):
                                po = psum_mm.tile([128, 512], F32, tag="mm",
                                                  name="pp")
                                nc.tensor.matmul(
                                    po[:],
                                    nts[p][:, tt * 128:(tt + 1) * 128],
                                    wpts[p][:, oc * 512:(oc + 1) * 512],
                                )
                                osl = oaccs[tt][:, oc * 512:(oc + 1) * 512]
                                if p == 0:
                                    nc.vector.tensor_copy(osl, po[:])
                                else:
                                    nc.vector.tensor_add(out=osl, in0=osl,
                                                         in1=po[:])
                            thunks.append(f)
                    return thunks

                for p in range(H // 2):
                    if p == 1:
                        for ct in range(CT_TILES):
                            nc.sync.dma_start(
                                out=wpts[ct][:],
                                in_=wpt[ct * 128:(ct + 1) * 128, :])
                    if p + 1 < H // 2:
                        nqt, nkt, nktd, nqtd, nthunks = prepare_qkv(p + 1)
                    else:
                        nqt = nkt = nktd = nqtd = None
                        nthunks = []
                    if p >= 1:
                        nthunks = nthunks + proj_partial_thunks(p - 1)
                    vp = vprimes[p % 2]
                    # interleave: one next-pair QKV chunk every few key-tiles
                    n_slots = 2 * KT_TILES  # kt iterations across both heads
                    sched = {}
                    for i, t in enumerate(nthunks):
                        sched.setdefault(
                            min(n_slots - 1, (i * n_slots) // max(1, len(nthunks))),
                            []).append(t)

                    with nc.named_scope(f"attn{p}"):
                        slot_i = 0
                        for h2 in range(2):
                            rb = h2 * 64
                            uacc = psum_u.tile([65, NQ], F32, tag="u",
                                               name=f"uacc{h2}")
                            for kt_idx in range(KT_TILES):
                                ps = psum_s.tile([128, NQ], F32, tag="s", name="pss")
                                ko = kt_idx * 128
                                for qc in range(NQ // 512):
                                    # every consecutive scores matmul flips row group
                                    ab = ((kt_idx * (NQ // 512) + qc) % 2) * 64
                                    qsl = slice(qc * 512, (qc + 1) * 512)
                                    nc.tensor.matmul(
                                        ps[:, qsl],
                                        ktd[h2][ab:ab + 64, ko:ko + 128],
                                        qtd[h2][ab:ab + 64, qsl],
                                    )
                                esb = exp_pool.tile([128, NQ], BF16, tag="e")
                                nc.scalar.activation(
                                    esb[:], ps[:],
                                    mybir.ActivationFunctionType.Exp, scale=SCALE)
                                slot = (h2 * KT_TILES + kt_idx) * VSLOT
                                for qc in range(NQ // 512):
                                    qsl = slice(qc * 512, (qc + 1) * 512)
                                    nc.tensor.matmul(
                                        uacc[:, qsl],
                                        vp[:, slot:slot + VSLOT],
                                        esb[:, qsl],
                                        start=(kt_idx == 0),
                                        stop=(kt_idx == KT_TILES - 1),
                                    )
                                for t in sched.get(slot_i, []):
                                    t()
                                slot_i += 1

                            # normalization for head h2
                            nt = nts[p]
                            # one copy drains the psum accumulator (frees the
                            # slot for the next head's AV immediately)
                            stg = rsb_pool.tile([65, NQ], BF16, tag="stg", name="stg")
                            nc.vector.tensor_copy(stg[:], uacc[:])
                            t8 = rsb_pool.tile([8, NQ // 8], BF16, tag="t8", name="t8")
                            nc.sync.dma_start(out=t8[:], in_=stg[64:65, :])
                            r8 = rsb_pool.tile([8, NQ // 8], BF16, tag="r8", name="r8")
                            with nc.allow_low_precision("bf16 matmul operand"):
                                nc.vector.reciprocal(r8[:], t8[:])
                            rsb = rsb_pool.tile([1, NQ], BF16, tag="r")
                            nc.sync.dma_start(out=rsb[:], in_=r8[:])
                            nc.vector.tensor_copy(nt[rb:rb + 64, :], stg[0:64, :])
                            for qc in range(NQ // 512):
                                pb = psum_mm.tile([128, 512], F32, tag="mm", name="pb")
                                nc.tensor.matmul(
                                    pb[0:64, :], ones[0:1, 0:64],
                                    rsb[0:1, qc * 512:(qc + 1) * 512],
                                )
                                nc.vector.tensor_mul(
                                    out=nt[rb:rb + 64, qc * 512:(qc + 1) * 512],
                                    in0=nt[rb:rb + 64, qc * 512:(qc + 1) * 512],
                                    in1=pb[0:64, :],
                                )
                    qt_sb, kt_sb, ktd, qtd = nqt, nkt, nktd, nqtd

            with contextlib.ExitStack() as es_proj:
                out_pool = es_proj.enter_context(tc.tile_pool(name="outp", bufs=3))
                with nc.named_scope("proj"):
                    for tt in range(NQ // 128):
                        for oc in range(C // 512):
                            po = psum_mm.tile([128, 512], F32, tag="mm", name="po")
                            nc.tensor.matmul(
                                po[:],
                                nts[CT_TILES - 1][:, tt * 128:(tt + 1) * 128],
                                wpts[CT_TILES - 1][:, oc * 512:(oc + 1) * 512],
                                start=True, stop=False,
                            )
                            nc.tensor.matmul(
                                po[:], ones[0:1, :],
                                bias_sb[0:1, oc * 512:(oc + 1) * 512],
                                start=False, stop=True,
                            )
                            ob = out_pool.tile([128, 512], F32, tag="ob")
                            nc.vector.tensor_add(
                                out=ob[:],
                                in0=oaccs[tt][:, oc * 512:(oc + 1) * 512],
                                in1=po[:],
                            )
                            nc.sync.dma_start(
                                out=out[tt * 128:(tt + 1) * 128,
                                        oc * 512:(oc + 1) * 512],
                                in_=ob[:],
                            )
    return nc


def make_in_maps(x, w_qkv, w_proj, b_proj):
    import ml_dtypes
    bf16 = ml_dtypes.bfloat16
    wqkvt = np.ascontiguousarray(w_qkv.T.astype(bf16))
    wpt = np.ascontiguousarray(w_proj.T.astype(bf16))
    bias = np.ascontiguousarray(np.asarray(b_proj).reshape(1, C).astype(bf16))
    in_maps = []
    for c in range(8):
        b, qh = c // 2, c % 2
        xb = x[b]
        xperm = np.concatenate(
            [xb[qh * NQ:(qh + 1) * NQ], xb[(1 - qh) * NQ:(2 - qh) * NQ]], axis=0)
        xt = np.ascontiguousarray(xperm.T.astype(bf16))
        in_maps.append({"xt": xt, "wqkvt": wqkvt, "wpt": wpt, "bias": bias})
    return in_maps


def assemble_output(results, x_shape):
    B, N, Cm = x_shape
    outp = np.empty((B, N, Cm), dtype=np.float32)
    for c in range(8):
        b, qh = c // 2, c % 2
        outp[b, qh * NQ:(qh + 1) * NQ, :] = results[c]["out"]
    return outp


_nc_cache = []


def kernel(x, w_qkv, w_proj, b_proj):
    from concourse.bass_utils import run_bass_kernel_spmd

    _apply_patches()
    x = np.asarray(x)
    if not _nc_cache:
        _nc_cache.append(build_nc())
    nc = _nc_cache[0]
    in_maps = make_in_maps(x, np.asarray(w_qkv), np.asarray(w_proj),
                           np.asarray(b_proj))
    res = run_bass_kernel_spmd(nc, in_maps, core_ids=list(range(8)))
    return assemble_output(res.results, (4, 2048, 1024)).astype(np.float32)

